# revision 1
# baseline (speedup 1.0000x reference)
"""Trainium2 Bass kernel for the 3-layer GNN message-passing model.

Strategy (8 NeuronCores, SPMD):
- Nodes dst-sharded: core c owns padded rows [c*12544, (c+1)*12544); edges live
  on their destination's core.
- Aggregation algebra is folded: segment_sum distributes over the linear layers,
  so the device scatter-adds raw gathered x[src] (and edge_attr once) and applies
  one folded [97,64] matmul per layer afterwards.  LayerNorm affine params fold
  into the next layer's weights, so the device only materializes the
  normalized z.
- Scatter-add runs on the TensorEngine: each destination gets K=16 fixed
  edge slots (mean degree), so the per-tile scatter matrices are 8 constant
  block patterns; overflow edges (~10%) use small data-driven one-hot matrices
  built on the VectorEngine.
- One launch per GNN layer (3 total).  Between launches the host re-shards the
  z output into per-core gathered edge streams (pure indexing).  Pooling +
  readout MLP run on-device in launch 3 with an AllReduce for the per-graph
  sums.
"""
import sys

sys.path.insert(0, "/opt/trn_rl_repo")

import numpy as np
from ml_dtypes import bfloat16

import concourse.bacc as bacc
import concourse.mybir as mybir
import concourse.tile as tile
from concourse.bass_utils import run_bass_kernel_spmd

NC = 8
N = 100000
E = 1600000
HID = 64
ED = 16
GD = 16
G = 64
NL = 3
EPS = 1e-5

NPC = 12500
SH = 12544            # 196 * 64 = 98 * 128
NPAD = NC * SH
BLK = 64
NBLK = SH // BLK      # 196
K = 16                # main slots per dst
TPB = BLK * K // 128  # 8 main tiles per block
NTMAIN = NBLK * TPB   # 1568
MAIN_ROWS = SH * K    # 200704
SLAB = 32             # main tiles per DMA slab
OVSLAB = 16           # overflow tiles per DMA slab

F32 = mybir.dt.float32
BF16 = mybir.dt.bfloat16

TRACE = False
DEBUG_L3 = False
EXEC_NS = []          # exec_time_ns per launch when TRACE
LAST_INSTS = []       # per-launch instruction lists when TRACE


# ----------------------------------------------------------------- host prep

def _pid_of(n):
    return (n // NPC) * SH + (n % NPC)


def _col_perm():
    """dst_local -> MT column. Within each 7-block (448-col) group, even
    blocks occupy cols 0:256 and odd blocks cols 256:448 (quadrant packing)."""
    return np.arange(SH, dtype=np.int64)


class _St:
    pass


def _build_structure(edge_index, batch):
    src = np.asarray(edge_index[0]).astype(np.int64)
    dst = np.asarray(edge_index[1]).astype(np.int64)
    st = _St()
    spid = _pid_of(src)

    order = np.lexsort((np.arange(E), dst))
    ds = dst[order]
    cnt = np.bincount(dst, minlength=N)
    starts = np.zeros(N + 1, np.int64)
    np.cumsum(cnt, out=starts[1:])
    ranks = np.arange(E) - starts[ds]
    eids = order
    dcore = ds // NPC
    dloc = ds % NPC

    main_mask = ranks < K
    st.main_src = np.full((NC, MAIN_ROWS), -1, np.int64)
    st.main_eid = np.full((NC, MAIN_ROWS), -1, np.int64)
    st.perm = _col_perm()
    st.iperm = np.argsort(st.perm)
    slot = st.perm[dloc] * K + ranks
    mc, msl = dcore[main_mask], slot[main_mask]
    msrc, meid = spid[eids[main_mask]], eids[main_mask]
    for c in range(NC):
        m = mc == c
        st.main_src[c, msl[m]] = msrc[m]
        st.main_eid[c, msl[m]] = meid[m]

    ov_mask = ~main_mask
    oc = dcore[ov_mask]
    ob = dloc[ov_mask] // BLK
    o_spid = spid[eids[ov_mask]]
    o_dloc = (dloc[ov_mask] % BLK).astype(np.float32)
    o_eid = eids[ov_mask]
    # position within (core, block) group
    key = oc * NBLK + ob
    korder = np.lexsort((np.arange(len(key)), key))
    ks = key[korder]
    gcnt = np.bincount(key, minlength=NC * NBLK)
    gstart = np.zeros(NC * NBLK + 1, np.int64)
    np.cumsum(gcnt, out=gstart[1:])
    grank = np.arange(len(ks)) - gstart[ks]
    # per-block overflow tile count = max over cores
    percb = gcnt.reshape(NC, NBLK)
    st.ovt = (percb.max(axis=0) + 127) // 128
    st.NOV = max(int(st.ovt.sum()) * 128, 128)
    st.NOVT = st.NOV // 128
    block_base = np.zeros(NBLK + 1, np.int64)
    np.cumsum(st.ovt * 128, out=block_base[1:])
    st.ov_block_of_tile = np.repeat(np.arange(NBLK), st.ovt)

    st.ov_src = np.full((NC, st.NOV), -1, np.int64)
    st.ov_eid = np.full((NC, st.NOV), -1, np.int64)
    st.ov_dloc = np.zeros((NC, st.NOV), np.float32)
    pos = block_base[ks % NBLK] + grank
    core_s = ks // NBLK
    sp_s = o_spid[korder]
    dl_s = o_dloc[korder]
    ei_s = o_eid[korder]
    for c in range(NC):
        m = core_s == c
        st.ov_src[c, pos[m]] = sp_s[m]
        st.ov_eid[c, pos[m]] = ei_s[m]
        st.ov_dloc[c, pos[m]] = dl_s[m]

    batch = np.asarray(batch).astype(np.int64)
    st.batch_pad = np.full(NPAD, -1, np.int64)
    for c in range(NC):
        bp = np.full(SH, -1, np.int64)
        bp[:NPC] = batch[c * NPC:(c + 1) * NPC]
        st.batch_pad[c * SH:(c + 1) * SH] = bp[st.iperm]   # column order
    st.cnt = np.bincount(batch, minlength=G).astype(np.float32)
    st.poh = np.zeros((NC, 128, SH // 128, G), np.float32)
    for c in range(NC):
        bp = st.batch_pad[c * SH:(c + 1) * SH].reshape(SH // 128, 128)
        t_idx, p_idx = np.nonzero(bp >= 0)
        st.poh[c, p_idx, t_idx, bp[t_idx, p_idx]] = 1.0
    return st


def _fold_weights(inp):
    W1, b1 = np.asarray(inp["W1"]), np.asarray(inp["b1"])
    W2, b2 = np.asarray(inp["W2"]), np.asarray(inp["b2"])
    Wu, bu = np.asarray(inp["Wu"]), np.asarray(inp["bu"])
    lnw, lnb = np.asarray(inp["lnw"]), np.asarray(inp["lnb"])
    out = []
    for i in range(NL):
        W12 = W1[i] @ W2[i][:HID]
        b12 = b1[i] @ W2[i][:HID] + b2[i]
        W2b = W2[i][HID:HID + ED]
        Wua = Wu[i][:HID]
        Wub = Wu[i][HID:HID + GD]
        lnw_p = np.ones(HID, np.float32) if i == 0 else lnw[i - 1]
        lnb_p = np.zeros(HID, np.float32) if i == 0 else lnb[i - 1]
        A = (np.diag(lnw_p) @ W12) @ Wua
        B = W2b @ Wua
        cvec = (lnb_p @ W12 + b12) @ Wua
        Wcat = np.concatenate(
            [A, B, cvec[None, :], np.zeros((15, HID), np.float32), Wub,
             bu[i][None, :]],
            0).astype(np.float32)
        out.append(np.ascontiguousarray(Wcat))
    return out


def _const_onehots():
    oh = np.zeros((128, 8), np.float32)
    for p in range(128):
        oh[p, p // K] = 1.0
    return oh


def _pad_x(x):
    xp = np.zeros((NPAD, HID), np.float32)
    for c in range(NC):
        xp[c * SH: c * SH + NPC] = x[c * NPC:(c + 1) * NPC]
    return xp


def _swz(a, w):
    """[rows, w] -> [128, (rows/128)*w] partition-major tile layout"""
    nt = a.shape[0] // 128
    return np.ascontiguousarray(
        a.reshape(nt, 128, w).transpose(1, 0, 2).reshape(128, nt * w))


def _gather_stream(st, xfull_pad, ea=None):
    """Swizzled per-core streams.  If ea is given (layer 0), interleave
    [x(64) | ea(16) | flag(1)] -> width 81, else width 64."""
    w = HID + ED + 1 if ea is not None else HID
    main = np.zeros((NC, 128, NTMAIN * w), bfloat16)
    ov = np.zeros((NC, 128, st.NOVT * w), bfloat16)
    for c in range(NC):
        mrow = np.zeros((MAIN_ROWS, w), np.float32)
        m = st.main_src[c] >= 0
        mrow[m, :HID] = xfull_pad[st.main_src[c][m]]
        if ea is not None:
            me = st.main_eid[c] >= 0
            mrow[me, HID:HID + ED] = ea[st.main_eid[c][me]]
            mrow[me, HID + ED] = 1.0
        main[c] = _swz(mrow, w).astype(bfloat16)
        orow = np.zeros((st.NOV, w), np.float32)
        mo = st.ov_src[c] >= 0
        orow[mo, :HID] = xfull_pad[st.ov_src[c][mo]]
        if ea is not None:
            moe = st.ov_eid[c] >= 0
            orow[moe, HID:HID + ED] = ea[st.ov_eid[c][moe]]
            orow[moe, HID + ED] = 1.0
        ov[c] = _swz(orow, w).astype(bfloat16)
    return main, ov


def _gpn_T(st, global_feature):
    gf = np.asarray(global_feature)
    out = np.zeros((NC, GD, SH), np.float32)
    for c in range(NC):
        bp = st.batch_pad[c * SH:(c + 1) * SH]
        m = bp >= 0
        out[c][:, m] = gf[bp[m]].T
    return out


# ------------------------------------------------------------ device program

def _build_layer_program(layer, novt, ov_block_of_tile):
    """layer: 0 (T-pass, z out, Tdeg out), 1 (Tdeg in, z out), 2 (Tdeg in,
    pooling + readout, scalar out)."""
    first = layer == 0
    last = layer == NL - 1
    EAW = ED + 1
    MW = HID + EAW if first else HID        # lhsT width (main & ov)
    PW = 81 if first else 64                # psum block partitions
    GRP = 7                                 # blocks per psum group
    NGRP = NBLK // GRP                      # 28
    GW = GRP * BLK                          # 448 psum columns

    nc = bacc.Bacc("TRN2", target_bir_lowering=False, debug=False)
    t_main = nc.dram_tensor("main", [128, NTMAIN * MW], BF16, kind="ExternalInput")
    t_ov = nc.dram_tensor("ov", [128, max(novt, 1) * MW], BF16, kind="ExternalInput")
    t_dstloc = nc.dram_tensor("dstloc", [128, max(novt, 1)], F32, kind="ExternalInput")
    t_oh8 = nc.dram_tensor("oh8", [128, 8], BF16, kind="ExternalInput")
    t_iota = nc.dram_tensor("iotar", [128, 8 * BLK], F32, kind="ExternalInput")
    t_gpnT = nc.dram_tensor("gpnT", [32, SH], F32, kind="ExternalInput")
    t_wcat = nc.dram_tensor("Wcat", [113, HID], F32, kind="ExternalInput")
    t_ident = nc.dram_tensor("ident", [128, 128], F32, kind="ExternalInput")
    if first:
        t_tdeg_out = nc.dram_tensor("TdegT", [17, SH], F32, kind="ExternalOutput")
    else:
        t_tdeg_in = nc.dram_tensor("TdegT_in", [17, SH], F32, kind="ExternalInput")
    if not last:
        t_z = nc.dram_tensor("z", [128, (SH // 128) * HID], F32, kind="ExternalOutput")
    else:
        t_poh = nc.dram_tensor("poh", [128, (SH // 128) * G], F32, kind="ExternalInput")
        t_scaleT = nc.dram_tensor("scaleT", [HID, G], F32, kind="ExternalInput")
        t_lnb3 = nc.dram_tensor("lnb3", [HID, 1], F32, kind="ExternalInput")
        t_gfT = nc.dram_tensor("gfT", [GD, G], F32, kind="ExternalInput")
        t_rw1 = nc.dram_tensor("rW1aug", [80, HID], F32, kind="ExternalInput")
        t_rb1t = nc.dram_tensor("rb1_t", [G, HID], F32, kind="ExternalInput")
        t_rln1w = nc.dram_tensor("rln1w_t", [G, HID], F32, kind="ExternalInput")
        t_rln1b = nc.dram_tensor("rln1b_t", [G, HID], F32, kind="ExternalInput")
        t_rw2 = nc.dram_tensor("rW2aug", [65, 32], F32, kind="ExternalInput")
        t_rln2w = nc.dram_tensor("rln2w_t", [G, 32], F32, kind="ExternalInput")
        t_rln2b = nc.dram_tensor("rln2b_t", [G, 32], F32, kind="ExternalInput")
        t_rw3 = nc.dram_tensor("rW3aug", [33, 1], F32, kind="ExternalInput")
        t_out = nc.dram_tensor("out", [G, 1], F32, kind="ExternalOutput")
        if DEBUG_L3:
            t_dsz = nc.dram_tensor("dbg_sz", [HID, G], F32, kind="ExternalOutput")
            t_dszf = nc.dram_tensor("dbg_szf", [HID, G], F32, kind="ExternalOutput")
            t_dr1 = nc.dram_tensor("dbg_r1", [G, HID], F32, kind="ExternalOutput")
            t_dr2 = nc.dram_tensor("dbg_r2", [G, 32], F32, kind="ExternalOutput")

    MT = nc.alloc_sbuf_tensor("MT", [113, SH], F32)

    ov_tiles_of_block = [[] for _ in range(NBLK)]
    for tt, b in enumerate(ov_block_of_tile):
        ov_tiles_of_block[b].append(tt)

    with tile.TileContext(nc) as tc:
        with (
            tc.tile_pool(name="const", bufs=1) as cp,
            tc.tile_pool(name="ms", bufs=4) as msp,
            tc.tile_pool(name="ovp", bufs=2) as ovp,
            tc.tile_pool(name="ohov", bufs=3) as ohp,
            tc.tile_pool(name="work", bufs=2) as wp,
            tc.tile_pool(name="psA", bufs=3, space="PSUM") as psA,
            tc.tile_pool(name="psB", bufs=2, space="PSUM") as psB,
            tc.tile_pool(name="psC", bufs=2, space="PSUM") as psC,
            tc.tile_pool(name="psD", bufs=1, space="PSUM") as psD,
        ):
            # ---- constants
            oh8_t = cp.tile([128, 8], BF16, tag="oh8")
            nc.sync.dma_start(out=oh8_t[:], in_=t_oh8[:])
            iota_t = cp.tile([128, 8 * BLK], F32, tag="iota")
            nc.sync.dma_start(out=iota_t[:], in_=t_iota[:])
            dstloc_t = cp.tile([128, max(novt, 1)], F32, tag="dstloc")
            nc.sync.dma_start(out=dstloc_t[:], in_=t_dstloc[:])
            wcat_t = cp.tile([113, HID], F32, tag="wcat")
            nc.sync.dma_start(out=wcat_t[:], in_=t_wcat[:])
            ident_t = cp.tile([128, 128], F32, tag="ident")
            nc.sync.dma_start(out=ident_t[:], in_=t_ident[:])
            eps_t = cp.tile([128, 1], F32, tag="eps")
            nc.vector.memset(eps_t[:], EPS)

            nc.sync.dma_start(out=MT[81:113, :], in_=t_gpnT[:])
            if not first:
                nc.sync.dma_start(out=MT[64:81, :], in_=t_tdeg_in[:])

            # ---- scatter phase
            nslab = (NTMAIN + SLAB - 1) // SLAB
            ms_tiles = [None] * nslab

            def load_main_slab(si):
                ntile = min(SLAB, NTMAIN - si * SLAB)
                sl = msp.tile([128, SLAB * MW], BF16, tag="ms")
                nc.sync.dma_start(
                    out=sl[:, 0:ntile * MW],
                    in_=t_main[:, si * SLAB * MW:(si * SLAB + ntile) * MW],
                )
                return sl

            novslab = (novt + OVSLAB - 1) // OVSLAB if novt else 0
            ov_tiles = [None] * max(novslab, 1)

            def load_ov_slab(si):
                ntile = min(OVSLAB, novt - si * OVSLAB)
                sl = ovp.tile([128, OVSLAB * MW], BF16, tag="ov")
                nc.sync.dma_start(
                    out=sl[:, 0:ntile * MW],
                    in_=t_ov[:, si * OVSLAB * MW:(si * OVSLAB + ntile) * MW],
                )
                return sl

            # batched overflow one-hots (8 tiles per DVE op)
            noh = (novt + 7) // 8 if novt else 0
            oh_tiles = [None] * max(noh, 1)

            def build_oh_batch(bi):
                ntile = min(8, novt - bi * 8)
                ohb = ohp.tile([128, 8, BLK], BF16, tag="oh")
                nc.vector.tensor_tensor(
                    out=ohb[:, 0:ntile, :],
                    in0=iota_t[:, 0:ntile * BLK].rearrange(
                        "p (k f) -> p k f", f=BLK),
                    in1=dstloc_t[:, bi * 8:bi * 8 + ntile].to_broadcast(
                        [128, ntile, BLK]),
                    op=mybir.AluOpType.is_equal,
                )
                return ohb

            quad = False
            PSP = PW if first else 128
            for gb in range(NGRP):
                ps = psA.tile([PSP, GW], F32, tag="blk")
                grp_has_ov = any(
                    ov_tiles_of_block[gb * GRP + bb2] for bb2 in range(GRP))
                # main MMs: 56 tiles, 8 columns each
                for tt in range(GRP * TPB):
                    tg = gb * GRP * TPB + tt
                    si, sj = tg // SLAB, tg % SLAB
                    if ms_tiles[si] is None:
                        ms_tiles[si] = load_main_slab(si)
                    par = 1 if (quad and tt >= 32) else 0
                    col0 = tt * 8
                    nc.tensor.matmul(
                        out=ps[64 * par:64 * par + PW, col0:col0 + 8],
                        lhsT=ms_tiles[si][:, sj * MW:sj * MW + MW],
                        rhs=oh8_t[:],
                        start=(tt == 0) or (quad and tt == 32),
                        stop=(tt == GRP * TPB - 1) and not grp_has_ov,
                        skip_group_check=True,
                        tile_position=(0, 64 * par) if quad else None,
                    )
                # overflow MMs
                for bb in range(GRP):
                    b = gb * GRP + bb
                    ovl = ov_tiles_of_block[b]
                    par = (bb % 2) if quad else 0
                    ocb = bb * BLK
                    for oi, tg in enumerate(ovl):
                        si, sj = tg // OVSLAB, tg % OVSLAB
                        if ov_tiles[si] is None:
                            ov_tiles[si] = load_ov_slab(si)
                        bi, bj = tg // 8, tg % 8
                        if oh_tiles[bi] is None:
                            oh_tiles[bi] = build_oh_batch(bi)
                        last_ov_bb = max(
                            b2 for b2 in range(GRP)
                            if ov_tiles_of_block[gb * GRP + b2])
                        is_last_ov = bb == last_ov_bb and oi == len(ovl) - 1
                        nc.tensor.matmul(
                            out=ps[64 * par:64 * par + PW, ocb:ocb + BLK],
                            lhsT=ov_tiles[si][:, sj * MW:sj * MW + MW],
                            rhs=oh_tiles[bi][:, bj, :],
                            start=False,
                            stop=is_last_ov,
                            skip_group_check=True,
                            tile_position=(0, 64 * par) if quad else None,
                        )
                if not quad:
                    nc.scalar.copy(
                        out=MT[0:PW, gb * GW:(gb + 1) * GW], in_=ps[0:PW, :])
                else:
                    g0 = gb * GW
                    nc.scalar.copy(
                        out=MT[0:64, g0:g0 + 256], in_=ps[0:64, 0:256])
                    nc.scalar.copy(
                        out=MT[0:64, g0 + 256:g0 + GW], in_=ps[64:128, 256:GW])

            if first:
                nc.sync.dma_start(out=t_tdeg_out[:], in_=MT[64:81, :])

            # ---- update + LN phase
            if last:
                poh_t = cp.tile([128, (SH // 128) * G], F32, tag="poh")
                nc.sync.dma_start(out=poh_t[:], in_=t_poh[:])
                ps_sumz = psD.tile([HID, G], F32, tag="ro")

            NJ = (SH + 511) // 512
            for j in range(NJ):
                n0 = j * 512
                nn_ = min(512, SH - n0)
                kt = nn_ // 128
                ph = psB.tile([HID, 512], F32, tag="hT")
                nc.tensor.matmul(
                    out=ph[:, :nn_], lhsT=wcat_t[:], rhs=MT[:, n0:n0 + nn_],
                    start=True, stop=True,
                )
                hts = wp.tile([HID, 512], F32, tag="hts")
                nc.scalar.copy(out=hts[:, :nn_], in_=ph[:, :nn_])
                ptr = psC.tile([128, 4 * HID], F32, tag="tr")
                for kk in range(kt):
                    nc.tensor.transpose(
                        out=ptr[:, kk * HID:(kk + 1) * HID],
                        in_=hts[:, kk * 128:(kk + 1) * 128],
                        identity=ident_t[0:HID, 0:HID],
                    )
                # LN over [128, kt, 64]
                ptr3 = ptr[:, 0:kt * HID].rearrange("p (k f) -> p k f", f=HID)
                mu = wp.tile([128, 4], F32, tag="mu")
                nc.vector.tensor_reduce(
                    out=mu[:, :kt], in_=ptr3, axis=mybir.AxisListType.X,
                    op=mybir.AluOpType.add,
                )
                nc.vector.tensor_scalar(
                    out=mu[:, :kt], in0=mu[:, :kt], scalar1=1.0 / HID,
                    scalar2=None, op0=mybir.AluOpType.mult,
                )
                tb = wp.tile([128, 4, HID], F32, tag="tb")
                nc.vector.tensor_tensor(
                    out=tb[:, :kt, :], in0=ptr3,
                    in1=mu[:, :kt].to_broadcast([128, kt, HID]),
                    op=mybir.AluOpType.subtract,
                )
                sq = wp.tile([128, 4, HID], F32, tag="sq")
                nc.vector.tensor_tensor(
                    out=sq[:, :kt, :], in0=tb[:, :kt, :], in1=tb[:, :kt, :],
                    op=mybir.AluOpType.mult,
                )
                var = wp.tile([128, 4], F32, tag="var")
                nc.vector.tensor_reduce(
                    out=var[:, :kt], in_=sq[:, :kt, :], axis=mybir.AxisListType.X,
                    op=mybir.AluOpType.add,
                )
                sd = wp.tile([128, 4], F32, tag="sd")
                nc.scalar.activation(
                    out=sd[:, :kt], in_=var[:, :kt],
                    func=mybir.ActivationFunctionType.Sqrt,
                    scale=1.0 / HID, bias=eps_t[:, 0:1],
                )
                rstd = wp.tile([128, 4], F32, tag="rstd")
                nc.vector.reciprocal(out=rstd[:, :kt], in_=sd[:, :kt])
                zb = wp.tile([128, 4, HID], F32, tag="zb")
                nc.vector.tensor_tensor(
                    out=zb[:, :kt, :], in0=tb[:, :kt, :],
                    in1=rstd[:, :kt].to_broadcast([128, kt, HID]),
                    op=mybir.AluOpType.mult,
                )
                if not last:
                    nc.sync.dma_start(
                        out=t_z[:, j * 4 * HID:(j * 4 + kt) * HID],
                        in_=zb[:, :kt, :],
                    )
                else:
                    for kk in range(kt):
                        tt = j * 4 + kk
                        nc.tensor.matmul(
                            out=ps_sumz[:],
                            lhsT=zb[:, kk, :],
                            rhs=poh_t[:, tt * G:(tt + 1) * G],
                            start=(tt == 0) or (quad and tt == 32),
                            stop=tt == SH // 128 - 1,
                        )

            # ---- readout (last layer only)
            if last:
                sz = wp.tile([HID, G], F32, tag="sz")
                nc.scalar.copy(out=sz[:], in_=ps_sumz[:])
                if DEBUG_L3:
                    nc.sync.dma_start(out=t_dsz[:], in_=sz[:])
                with tc.tile_pool(name="dram", bufs=1, space="DRAM") as dp:
                    arin = dp.tile([HID, G], F32)
                    arout = dp.tile([HID, G], F32)
                    nc.gpsimd.dma_start(arin[:], sz[:])
                    nc.gpsimd.collective_compute(
                        "AllReduce",
                        mybir.AluOpType.add,
                        replica_groups=[list(range(NC))],
                        ins=[arin.opt()],
                        outs=[arout.opt()],
                    )
                    szf = wp.tile([HID, G], F32, tag="szf")
                    nc.gpsimd.dma_start(szf[:], arout[:])
                if DEBUG_L3:
                    nc.sync.dma_start(out=t_dszf[:], in_=szf[:])

                scaleT_t = cp.tile([HID, G], F32, tag="scaleT")
                nc.sync.dma_start(out=scaleT_t[:], in_=t_scaleT[:])
                lnb3_t = cp.tile([HID, 1], F32, tag="lnb3")
                nc.sync.dma_start(out=lnb3_t[:], in_=t_lnb3[:])
                gfT_t = cp.tile([GD, G], F32, tag="gfT")
                nc.sync.dma_start(out=gfT_t[:], in_=t_gfT[:])
                rw1_t = cp.tile([80, HID], F32, tag="rw1")
                nc.sync.dma_start(out=rw1_t[:], in_=t_rw1[:])
                rb1t_t = cp.tile([G, HID], F32, tag="rb1t")
                nc.sync.dma_start(out=rb1t_t[:], in_=t_rb1t[:])
                rln1w_t = cp.tile([G, HID], F32, tag="rln1w")
                nc.sync.dma_start(out=rln1w_t[:], in_=t_rln1w[:])
                rln1b_t = cp.tile([G, HID], F32, tag="rln1b")
                nc.sync.dma_start(out=rln1b_t[:], in_=t_rln1b[:])
                rw2_t = cp.tile([65, 32], F32, tag="rw2")
                nc.sync.dma_start(out=rw2_t[:], in_=t_rw2[:])
                rln2w_t = cp.tile([G, 32], F32, tag="rln2w")
                nc.sync.dma_start(out=rln2w_t[:], in_=t_rln2w[:])
                rln2b_t = cp.tile([G, 32], F32, tag="rln2b")
                nc.sync.dma_start(out=rln2b_t[:], in_=t_rln2b[:])
                rw3_t = cp.tile([33, 1], F32, tag="rw3")
                nc.sync.dma_start(out=rw3_t[:], in_=t_rw3[:])

                inT = wp.tile([80, G], F32, tag="inT")
                nc.vector.tensor_tensor(
                    out=inT[0:HID, :], in0=szf[:], in1=scaleT_t[:],
                    op=mybir.AluOpType.mult,
                )
                nc.vector.tensor_scalar(
                    out=inT[0:HID, :], in0=inT[0:HID, :],
                    scalar1=lnb3_t[:, 0:1], scalar2=None,
                    op0=mybir.AluOpType.add,
                )
                nc.vector.tensor_copy(out=inT[HID:HID + GD, :], in_=gfT_t[:])

                def ln_affine_relu(ph_in, width, w_t, b_t, out_tile):
                    mu1 = wp.tile([G, 1], F32, tag="rmu")
                    nc.vector.tensor_reduce(
                        out=mu1[:], in_=ph_in, axis=mybir.AxisListType.X,
                        op=mybir.AluOpType.add,
                    )
                    nc.vector.tensor_scalar(
                        out=mu1[:], in0=mu1[:], scalar1=1.0 / width,
                        scalar2=None, op0=mybir.AluOpType.mult,
                    )
                    tb1 = wp.tile([G, width], F32, tag=f"rtb{width}")
                    nc.vector.tensor_scalar(
                        out=tb1[:], in0=ph_in, scalar1=mu1[:, 0:1],
                        scalar2=None, op0=mybir.AluOpType.subtract,
                    )
                    sq1 = wp.tile([G, width], F32, tag=f"rsq{width}")
                    nc.vector.tensor_tensor(
                        out=sq1[:], in0=tb1[:], in1=tb1[:],
                        op=mybir.AluOpType.mult,
                    )
                    var1 = wp.tile([G, 1], F32, tag="rvar")
                    nc.vector.tensor_reduce(
                        out=var1[:], in_=sq1[:], axis=mybir.AxisListType.X,
                        op=mybir.AluOpType.add,
                    )
                    sd1 = wp.tile([G, 1], F32, tag="rsd")
                    nc.scalar.activation(
                        out=sd1[:], in_=var1[:],
                        func=mybir.ActivationFunctionType.Sqrt,
                        scale=1.0 / width, bias=eps_t[0:G, 0:1],
                    )
                    rstd1 = wp.tile([G, 1], F32, tag="rrstd")
                    nc.vector.reciprocal(out=rstd1[:], in_=sd1[:])
                    nc.vector.tensor_scalar(
                        out=tb1[:], in0=tb1[:], scalar1=rstd1[:, 0:1],
                        scalar2=None, op0=mybir.AluOpType.mult,
                    )
                    nc.vector.tensor_tensor(
                        out=tb1[:], in0=tb1[:], in1=w_t[:],
                        op=mybir.AluOpType.mult,
                    )
                    nc.vector.tensor_tensor(
                        out=tb1[:], in0=tb1[:], in1=b_t[:],
                        op=mybir.AluOpType.add,
                    )
                    nc.vector.tensor_scalar(
                        out=out_tile[:], in0=tb1[:], scalar1=0.0,
                        scalar2=None, op0=mybir.AluOpType.max,
                    )

                ph1 = psD.tile([G, HID], F32, tag="ro")
                nc.tensor.matmul(out=ph1[:], lhsT=inT[:], rhs=rw1_t[:],
                                 start=True, stop=True)
                h1s = wp.tile([G, HID], F32, tag="h1s")
                nc.vector.tensor_tensor(out=h1s[:], in0=ph1[:], in1=rb1t_t[:],
                                        op=mybir.AluOpType.add)
                r1 = wp.tile([G, HID], F32, tag="r1")
                ln_affine_relu(h1s[:], HID, rln1w_t, rln1b_t, r1)

                if DEBUG_L3:
                    nc.sync.dma_start(out=t_dr1[:], in_=r1[:])
                ptr1 = psD.tile([G, HID], F32, tag="ro")
                nc.tensor.transpose(out=ptr1[:], in_=r1[:],
                                    identity=ident_t[0:G, 0:G])
                inT2 = wp.tile([65, G], F32, tag="inT2")
                nc.scalar.copy(out=inT2[0:HID, :], in_=ptr1[:])
                nc.vector.memset(inT2[64:65, :], 1.0)

                ph2 = psD.tile([G, 32], F32, tag="ro")
                nc.tensor.matmul(out=ph2[:], lhsT=inT2[:], rhs=rw2_t[:],
                                 start=True, stop=True)
                r2 = wp.tile([G, 32], F32, tag="r2")
                ln_affine_relu(ph2[:], 32, rln2w_t, rln2b_t, r2)

                if DEBUG_L3:
                    nc.sync.dma_start(out=t_dr2[:], in_=r2[:])
                ptr2 = psD.tile([32, G], F32, tag="ro")
                nc.tensor.transpose(out=ptr2[:], in_=r2[:],
                                    identity=ident_t[0:G, 0:G])
                inT3 = wp.tile([33, G], F32, tag="inT3")
                nc.scalar.copy(out=inT3[0:32, :], in_=ptr2[:])
                nc.vector.memset(inT3[32:33, :], 1.0)

                ph3 = psD.tile([G, 1], F32, tag="ro")
                nc.tensor.matmul(out=ph3[:], lhsT=inT3[:], rhs=rw3_t[:],
                                 start=True, stop=True)
                outs = wp.tile([G, 1], F32, tag="outs")
                nc.scalar.copy(out=outs[:], in_=ph3[:])
                nc.sync.dma_start(out=t_out[:], in_=outs[:])

    nc.compile()
    return nc


# ------------------------------------------------------------------- driver

def kernel(**inputs):
    global EXEC_NS, LAST_INSTS
    EXEC_NS = []
    LAST_INSTS = []
    inputs = {k: np.asarray(v) for k, v in inputs.items()}
    st = _build_structure(inputs["edge_index"], inputs["batch"])
    folded = _fold_weights(inputs)
    oh8 = _const_onehots().astype(bfloat16)
    iotar = np.tile(np.arange(BLK, dtype=np.float32), (128, 8))
    gpnT = _gpn_T(st, inputs["global_feature"])
    gpnT_aug = np.concatenate(
        [np.zeros((NC, 15, SH), np.float32), gpnT,
         np.ones((NC, 1, SH), np.float32)], axis=1)  # rows 15:31 gpn, 31 ones
    xpad = _pad_x(inputs["x"].astype(np.float32))

    lnw3 = np.asarray(inputs["lnw"])[2].astype(np.float32)
    lnb3 = np.asarray(inputs["lnb"])[2].astype(np.float32)
    scaleT = np.ascontiguousarray(lnw3[:, None] / np.maximum(st.cnt, 1.0)[None, :]).astype(np.float32)
    gfT = np.ascontiguousarray(np.asarray(inputs["global_feature"]).T.astype(np.float32))
    rW1aug = np.asarray(inputs["rW1"]).astype(np.float32)
    rb1_t = np.tile(inputs["rb1"].astype(np.float32), (G, 1))
    rW2aug = np.concatenate([inputs["rW2"], inputs["rb2"][None, :]], 0).astype(np.float32)
    rW3aug = np.concatenate([inputs["rW3"], inputs["rb3"][None, :]], 0).astype(np.float32)
    rln1w_t = np.tile(inputs["rln1w"].astype(np.float32), (G, 1))
    rln1b_t = np.tile(inputs["rln1b"].astype(np.float32), (G, 1))
    rln2w_t = np.tile(inputs["rln2w"].astype(np.float32), (G, 1))
    rln2b_t = np.tile(inputs["rln2b"].astype(np.float32), (G, 1))
    ident = np.eye(128, dtype=np.float32)

    TdegT = None
    xcur = xpad
    ea = inputs["edge_attr"].astype(np.float32)
    for layer in range(NL):
        Wcat = folded[layer]
        prog = _build_layer_program(layer, st.NOVT, st.ov_block_of_tile)
        main, ov = _gather_stream(st, xcur, ea=ea if layer == 0 else None)
        in_maps = []
        for c in range(NC):
            im = {
                "main": main[c],
                "ov": ov[c],
                "dstloc": np.ascontiguousarray(
                    st.ov_dloc[c].reshape(st.NOVT, 128).T),
                "oh8": oh8,
                "iotar": iotar,
                "gpnT": np.ascontiguousarray(gpnT_aug[c]),
                "Wcat": Wcat,
                "ident": ident,
            }
            if layer != 0:
                im["TdegT_in"] = np.ascontiguousarray(TdegT[c])
            if layer == NL - 1:
                im["poh"] = np.ascontiguousarray(
                    st.poh[c].reshape(128, (SH // 128) * G))
                im["scaleT"] = scaleT
                im["lnb3"] = np.ascontiguousarray(lnb3[:, None])
                im["gfT"] = gfT
                im["rW1aug"] = rW1aug
                im["rb1_t"] = rb1_t
                im["rln1w_t"] = rln1w_t
                im["rln1b_t"] = rln1b_t
                im["rW2aug"] = rW2aug
                im["rln2w_t"] = rln2w_t
                im["rln2b_t"] = rln2b_t
                im["rW3aug"] = rW3aug
            in_maps.append(im)
        res = run_bass_kernel_spmd(
            prog, in_maps, core_ids=list(range(NC)), trace=TRACE)
        if TRACE:
            EXEC_NS.append(res.exec_time_ns)
            LAST_INSTS.append(res.instructions_and_trace[0] if res.instructions_and_trace else None)
        if layer == 0:
            TdegT = [res.results[c]["TdegT"] for c in range(NC)]
        if layer < NL - 1:
            xnext = np.zeros((NPAD, HID), np.float32)
            for c in range(NC):
                zsw = res.results[c]["z"]          # [128, 98*64]
                zcol = (zsw.reshape(128, SH // 128, HID)
                        .transpose(1, 0, 2).reshape(SH, HID))
                xnext[c * SH:(c + 1) * SH] = zcol[st.perm]   # back to dl order
            xcur = xnext
        else:
            out = res.results[0]["out"][:, 0].astype(np.float32)
    return out

